# revision 1
# baseline (speedup 1.0000x reference)
"""Trainium2 Bass kernel for nn_AttentionHead (B=4, S=2048, DK=1024).

Single-head attention with input projections:
    qp = q @ wq.T; kp = k @ wk.T; vp = v @ wv.T
    s  = qp @ kp.T / sqrt(dk); attn = softmax(s); out = attn @ vp

Sharding: 8 cores = (batch b in 0..3) x (query-row half h in 0..1).
Each core computes the full K/V projection for its batch (duplicated
across the pair) and attention for its 1024 query rows.

Device-side layout trick: everything is kept "feature-major" so all
matmul contractions land on the partition dim with zero on-device
transposes. The host passes q/k/v/w pre-transposed; the kernel returns
out.T per core and the host transposes back.

Per core:
    kpT[e,j] = sum_d wkT[d,e] * kT[d,j]      (256 MMs)
    qpT[e,i] = sum_d wqT[d,e] * qT[d,i]      (128 MMs)
    sT[j,i]  = sum_e kpT[e,j] * qpT[e,i]     (256 MMs)
    eT[j,i]  = exp(sT/32)                     (ACT, fused scale; round-trips
                                               through DRAM to free SBUF)
    cs[i]    = sum_j eT[j,i]  via ones-matmul (broadcast over partitions)
    vp[j,e]  = sum_d vT[d,j] * wvT[d,e]      (256 MMs)
    outT[e,i]= (sum_j vp[j,e] * eT[j,i]) * (1/cs[i])   (256 MMs)

Matmuls run as float32r (fp32 bytes, single-pass PE mode, ~4x the
fp32 rate). All matmul operands are produced directly in float32r
(DMA loads and engine writes), satisfying the BIR verifier's
"rounded to FP32r" rule. Measured end-to-end relative error vs the
fp32 reference: ~4e-4.

SBUF budget is ~208KB/partition, managed as two allocation stacks
(left/right) with phase-scoped pools. Inputs stream through small
rotating chunk pools ([128,512] tiles, 2 slots per contraction tile)
in first-use order so DMA overlaps compute; 52 warm-up matmuls on a
constant tile keep the PE HAM clock at full rate while the first
input chunks land. Colsum matmuls trail their exp by one group so
the in-order PE never waits on the ACT engine.

exp(sT) round-trips through DRAM (staged exp tiles DMA out during
the score phase, streamed back in i-slice halves with a split-j
accumulation in the output phase). That frees 64KB of SBUF, which
lets wv prefetch during earlier phases via the weight-pool rotation
— the PE runs gap-free from warm-up to the last matmul and the HAM
clock stays at 2.4GHz for the whole kernel.

Measured on 8 axon-attached TRN2 cores: ~304 us HW exec time
(PE-limited; 1184 N=512 fp32r matmuls/core stream at ~233 ns each;
phases A-F all within ~2% of the matmul issue-rate floor).
"""

import numpy as np

_B, _S, _DK = 4, 2048, 1024
_HALF = _S // 2
_N_CORES = 8
_P = 128

_CACHE = {}


def _emit(tc, qT, kT, vT, wqT, wkT, wvT, outT, DK, S, HALF, mm_dt):
    import concourse.bass as bass
    from concourse import mybir

    nc = tc.nc
    ts = bass.ts
    P = _P
    NF = min(512, HALF, S, DK)
    DT = DK // P        # contraction tiles (d)
    ET = DK // P        # output-feature tiles (e)
    JT = S // P         # key tiles (j)
    ISL = HALF // NF    # query slices (i)
    JSL = S // NF       # key slices
    ESL = DK // NF      # feature slices
    JGN = S // NF       # vT chunk groups (NF//P j-tiles each)
    JPG = NF // P       # j-tiles per vT chunk
    NORM = 1.0 / float(np.sqrt(DK))
    f32 = mybir.dt.float32
    AF = mybir.ActivationFunctionType

    _cms = {}

    def opn(**kw):
        cm = tc.tile_pool(**kw)
        pool = cm.__enter__()
        _cms[id(pool)] = cm
        return pool

    def cls(*pools):
        for pool in pools:
            _cms.pop(id(pool)).__exit__(None, None, None)

    # ---------------- pools ----------------
    # LEFT stack: misc | x (stream rotation) | kpT | qpT | later vp, wv
    # RIGHT stack: stage | w (wk/wq chunks) | later eT
    misc = opn(name="misc", bufs=1, side="left")
    xp = opn(name="xp", bufs=1, side="left")
    stage = opn(name="stage", bufs=2, side="right")
    wp = opn(name="wp", bufs=1, side="right")
    psmm = opn(name="psmm", bufs=6, space="PSUM")
    psacc = opn(name="psacc", bufs=1, space="PSUM")
    dram = opn(name="dram", bufs=1, space="DRAM")
    eT_dram = dram.tile([S, HALF], mm_dt, name="et_dram")

    ones_f32 = misc.tile([P, P], f32, tag="ones_f32")
    nc.vector.memset(ones_f32[:], 1.0)
    ones = misc.tile([P, P], mm_dt, tag="ones")
    nc.vector.tensor_copy(ones[:], ones_f32[:])
    recip = misc.tile([P, HALF], f32, tag="recip")
    cs_ps = [psacc.tile([P, NF], f32, tag=f"cs{i}", name=f"cs{i}") for i in range(ISL)]

    # x-pool rotation: per-d stream chunks [P, NF], 2 slots.
    # Allocation order per d: k[0..JSL-1], q[0..ISL-1], vs[0..JGN-1].
    def x_tile(kind, d, idx):
        return xp.tile([P, NF], mm_dt, tag=f"x{d}", bufs=2, name=f"{kind}{idx}_d{d}")

    # ---------------- PE warm-up while first DMAs land ----------------
    warm_ps = psmm.tile([P, P], f32, tag="mm", name="warm_ps")
    for _ in range(52):
        nc.tensor.matmul(warm_ps[:], ones[:], ones[:], start=True, stop=True)

    # ---------------- phase A: kpT = (k @ wk.T).T ----------------
    kp_pool = opn(name="kpp", bufs=1, side="left")
    kpT = [kp_pool.tile([P, S], mm_dt, tag=f"kp{e}", name=f"kp{e}") for e in range(ET)]

    # wk/wq chunk slots [P, NF] (e-halves), 2 bufs: slot0 = wk, slot1 = wq
    EPC = NF // P  # e-tiles per w chunk
    WH = ET // EPC  # w chunks per d
    wk_c = [[None] * WH for _ in range(DT)]
    wq_c = [[None] * WH for _ in range(DT)]

    def load_w(dst, d, h, src, nm, eng=None):
        t = wp.tile([P, NF], mm_dt, tag=f"w{d}h{h}", bufs=2, name=f"{nm}{d}_{h}")
        (eng or nc.sync).dma_start(t[:], src[ts(d, P), ts(h, NF)])
        dst[d][h] = t

    def w_slice(c, d, e):
        return c[d][e // EPC][:, ts(e % EPC, P)]

    k_c = [[None] * JSL for _ in range(DT)]
    q_c = [[None] * ISL for _ in range(DT)]
    vs_c = [[None] * JGN for _ in range(DT)]

    # first-use-ordered input streaming: wk[*][h0], k[*][js0], wk[*][h1..]
    for d in range(DT):
        load_w(wk_c, d, 0, wkT, "wk")
    for d in range(DT):
        k_c[d][0] = x_tile("k", d, 0)
        nc.sync.dma_start(k_c[d][0][:], kT[ts(d, P), ts(0, NF)])
    for h in range(1, WH):
        for d in range(DT):
            load_w(wk_c, d, h, wkT, "wk")

    for js in range(JSL):
        if js + 1 < JSL:  # prefetch next k slab
            for d in range(DT):
                k_c[d][js + 1] = x_tile("k", d, js + 1)
                nc.sync.dma_start(k_c[d][js + 1][:], kT[ts(d, P), ts(js + 1, NF)])
        if js == min(1, JSL - 1):  # wq loads ride behind early k prefetches
            for h in range(WH):
                for d in range(DT):
                    load_w(wq_c, d, h, wqT, "wq")
        if js == min(2, JSL - 1):  # q[isl0] into freed k slots
            for d in range(DT):
                q_c[d][0] = x_tile("q", d, 0)
                nc.sync.dma_start(q_c[d][0][:], qT[ts(d, P), ts(0, NF)])
        if js == JSL - 1:  # remaining q slices
            for isl in range(1, ISL):
                for d in range(DT):
                    q_c[d][isl] = x_tile("q", d, isl)
                    nc.sync.dma_start(q_c[d][isl][:], qT[ts(d, P), ts(isl, NF)])
        for e in range(ET):
            ps = psmm.tile([P, NF], f32, tag="mm")
            for d in range(DT):
                nc.tensor.matmul(
                    ps[:],
                    w_slice(wk_c, d, e),
                    k_c[d][js][:],
                    start=(d == 0),
                    stop=(d == DT - 1),
                )
            nc.vector.tensor_copy(kpT[e][:, ts(js, NF)], ps[:])

    # ---------------- phase B: qpT = (q @ wq.T).T ----------------
    qp_pool = opn(name="qpp", bufs=1, side="left")
    qpT = [
        qp_pool.tile([P, HALF], mm_dt, tag=f"qp{e}", name=f"qp{e}") for e in range(ET)
    ]
    for isl in range(ISL):
        for e in range(ET):
            ps = psmm.tile([P, NF], f32, tag="mm")
            for d in range(DT):
                nc.tensor.matmul(
                    ps[:],
                    w_slice(wq_c, d, e),
                    q_c[d][isl][:],
                    start=(d == 0),
                    stop=(d == DT - 1),
                )
            nc.vector.tensor_copy(qpT[e][:, ts(isl, NF)], ps[:])
    # wv rides the w-rotation (slot freed when wk releases at end of A),
    # so it loads during B/C — no stall at the C->E boundary.
    wv_c = [[None] * WH for _ in range(DT)]
    for h in range(WH):
        for d in range(DT):
            load_w(wv_c, d, h, wvT, "wv")

    # ---------------- phase C: sT -> exp -> eT_dram (+ colsum), vs prefetch ----
    etsp = opn(name="etsp", bufs=1, side="right")
    # vs chunks 0/1 drain as soon as q slots free (mid/end of phase B)
    for g in range(min(2, JGN)):
        for d in range(DT):
            vs_c[d][g] = x_tile("vs", d, g)
            nc.sync.dma_start(vs_c[d][g][:], vT[ts(d, P), ts(g, NF)])
    pending_cs = []
    for j in range(JT):
        for isl in range(ISL):
            ps = psmm.tile([P, NF], f32, tag="mm")
            for e in range(ET):
                nc.tensor.matmul(
                    ps[:],
                    kpT[e][:, ts(j, P)],
                    qpT[e][:, ts(isl, NF)],
                    start=(e == 0),
                    stop=(e == ET - 1),
                )
            st = etsp.tile([P, NF], mm_dt, tag="ets", bufs=3, name=f"ets{j}_{isl}")
            nc.scalar.activation(st[:], ps[:], AF.Exp, scale=NORM)
            nc.sync.dma_start(eT_dram[ts(j, P), ts(isl, NF)], st[:])
            pending_cs.append((j, isl, st))
            if len(pending_cs) > 1:
                pj, pisl, pst = pending_cs.pop(0)
                nc.tensor.matmul(
                    cs_ps[pisl][:],
                    ones[:],
                    pst[:],
                    start=(pj == 0),
                    stop=(pj == JT - 1),
                )
    for pj, pisl, pst in pending_cs:
        nc.tensor.matmul(
            cs_ps[pisl][:],
            ones[:],
            pst[:],
            start=(pj == 0),
            stop=(pj == JT - 1),
        )
    for isl in range(ISL):
        nc.vector.reciprocal(recip[:, ts(isl, NF)], cs_ps[isl][:])
    cls(etsp)
    cls(qp_pool, kp_pool)
    cls(psacc)

    # ---------------- phase E: vp = v @ wv.T ----------------
    vp_pool = opn(name="vpp", bufs=1, side="left")
    vp = [vp_pool.tile([P, DK], mm_dt, tag=f"vp{j}", name=f"vp{j}") for j in range(JT)]
    # eT comes back from DRAM in halves during E/F (tag rotation per j%8)
    ethp = opn(name="ethp", bufs=1, side="left")
    eth = [[None] * JT for _ in range(ISL)]

    def load_eth(isl, jlist):
        for j in jlist:
            t = ethp.tile(
                [P, NF], mm_dt, tag=f"eh{j % 8}", bufs=2, name=f"eh{isl}_{j}"
            )
            nc.sync.dma_start(t[:], eT_dram[ts(j, P), ts(isl, NF)])
            eth[isl][j] = t

    load_eth(0, range(JT // 2))
    load_eth(0, range(JT // 2, JT))
    for g in range(JGN):
        if g + 2 < JGN:  # double-buffered vs prefetch
            gg = g + 2
            for d in range(DT):
                vs_c[d][gg] = x_tile("vs", d, gg)
                nc.sync.dma_start(vs_c[d][gg][:], vT[ts(d, P), ts(gg, NF)])
        for jin in range(JPG):
            j = g * JPG + jin
            for es in range(ESL):
                ps = psmm.tile([P, NF], f32, tag="mm")
                for d in range(DT):
                    nc.tensor.matmul(
                        ps[:],
                        vs_c[d][g][:, ts(jin, P)],
                        wv_c[d][es][:],
                        start=(d == 0),
                        stop=(d == DT - 1),
                    )
                nc.vector.tensor_copy(vp[j][:, ts(es, NF)], ps[:])
    cls(wp)
    cls(psmm)

    # ---------------- phase F: outT = (eT.T @ vp).T * recip ----------------
    # Two j-half passes per i-slice so eth tiles release mid-slice and the
    # next slice's eth loads prefetch without a stall. One PSUM bank per e.
    pf = opn(name="pf", bufs=1, space="PSUM")
    JH = JT // 2
    for isl in range(ISL):
        pft = [
            pf.tile([P, NF], f32, tag=f"pf{e}", name=f"pf{e}_{isl}")
            for e in range(ET)
        ]
        for e in range(ET):
            for j in range(JH):
                nc.tensor.matmul(
                    pft[e][:],
                    vp[j][:, ts(e, P)],
                    eth[isl][j][:],
                    start=(j == 0),
                    stop=False,
                )
        if isl + 1 < ISL:
            load_eth(isl + 1, range(JH))
        for e in range(ET):
            for j in range(JH, JT):
                nc.tensor.matmul(
                    pft[e][:],
                    vp[j][:, ts(e, P)],
                    eth[isl][j][:],
                    start=False,
                    stop=(j == JT - 1),
                )
            ot = stage.tile([P, NF], f32, tag="ost")
            nc.vector.tensor_mul(ot[:], pft[e][:], recip[:, ts(isl, NF)])
            nc.sync.dma_start(outT[ts(e, P), ts(isl, NF)], ot[:])
        if isl + 1 < ISL:
            load_eth(isl + 1, range(JH, JT))
    cls(ethp, vp_pool, xp, misc, stage, pf, dram)


def build_program(DK=_DK, S=_S, HALF=_HALF, mm_dtype="float32r"):
    """Build + compile the per-core Bass program. Returns the Bacc object."""
    import concourse.tile as tile
    from concourse import bacc, mybir

    f32 = mybir.dt.float32
    mm_dt = getattr(mybir.dt, mm_dtype)

    nc = bacc.Bacc(
        "TRN2",
        target_bir_lowering=False,
        debug=False,
        enable_asserts=False,
        num_devices=_N_CORES,
    )
    qT = nc.dram_tensor("qt", (DK, HALF), mm_dt, kind="ExternalInput").ap()
    kT = nc.dram_tensor("kt", (DK, S), mm_dt, kind="ExternalInput").ap()
    vT = nc.dram_tensor("vt", (DK, S), mm_dt, kind="ExternalInput").ap()
    wqT = nc.dram_tensor("wqt", (DK, DK), mm_dt, kind="ExternalInput").ap()
    wkT = nc.dram_tensor("wkt", (DK, DK), mm_dt, kind="ExternalInput").ap()
    wvT = nc.dram_tensor("wvt", (DK, DK), mm_dt, kind="ExternalInput").ap()
    outT = nc.dram_tensor("outt", (DK, HALF), f32, kind="ExternalOutput").ap()

    with tile.TileContext(nc) as tc:
        _emit(tc, qT, kT, vT, wqT, wkT, wvT, outT, DK, S, HALF, mm_dt)
    nc.compile()
    return nc


def _in_maps(q, k, v, wq, wk, wv):
    """Shard full inputs into 8 per-core input maps (host-side transposes)."""
    wqT = np.ascontiguousarray(wq.T)
    wkT = np.ascontiguousarray(wk.T)
    wvT = np.ascontiguousarray(wv.T)
    kT_b = [np.ascontiguousarray(k[b].T) for b in range(_B)]
    vT_b = [np.ascontiguousarray(v[b].T) for b in range(_B)]
    maps = []
    for c in range(_N_CORES):
        b, h = divmod(c, 2)
        qT = np.ascontiguousarray(q[b, h * _HALF : (h + 1) * _HALF, :].T)
        maps.append(
            {
                "qt": qT,
                "kt": kT_b[b],
                "vt": vT_b[b],
                "wqt": wqT,
                "wkt": wkT,
                "wvt": wvT,
            }
        )
    return maps


def kernel(q, k, v, wq, wk, wv):
    from concourse.bass_utils import run_bass_kernel_spmd

    q = np.asarray(q, np.float32)
    k = np.asarray(k, np.float32)
    v = np.asarray(v, np.float32)
    wq = np.asarray(wq, np.float32)
    wk = np.asarray(wk, np.float32)
    wv = np.asarray(wv, np.float32)

    if "nc" not in _CACHE:
        _CACHE["nc"] = build_program()
    nc = _CACHE["nc"]

    res = run_bass_kernel_spmd(
        nc, _in_maps(q, k, v, wq, wk, wv), core_ids=list(range(_N_CORES))
    )

    out = np.empty((_B, _S, _DK), np.float32)
    for c in range(_N_CORES):
        b, h = divmod(c, 2)
        out[b, h * _HALF : (h + 1) * _HALF, :] = res.results[c]["outt"].T
    return out



# revision 3
# speedup vs baseline: 1.0887x; 1.0887x over previous
"""Trainium2 Bass kernel for nn_AttentionHead (B=4, S=2048, DK=1024).

Single-head attention with input projections:
    qp = q @ wq.T; kp = k @ wk.T; vp = v @ wv.T
    s  = qp @ kp.T / sqrt(dk); attn = softmax(s); out = attn @ vp

Sharding: 8 cores = (batch b in 0..3) x (query-row half h in 0..1).
Each core computes the full K/V projection for its batch (duplicated
across the pair) and attention for its 1024 query rows.

Device-side layout trick: everything is kept "feature-major" so all
matmul contractions land on the partition dim with zero on-device
transposes. The host passes q/k/v/w pre-transposed; the kernel returns
out.T per core and the host transposes back.

Per core:
    kpT[e,j] = sum_d wkT[d,e] * kT[d,j]      (256 MMs)
    qpT[e,i] = sum_d wqT[d,e] * qT[d,i]      (128 MMs)
    sT[j,i]  = sum_e kpT[e,j] * qpT[e,i]     (256 MMs)
    eT[j,i]  = exp(sT/32)                     (ACT, fused scale; round-trips
                                               through DRAM to free SBUF)
    cs[i]    = sum_j eT[j,i]  via ones-matmul (broadcast over partitions)
    vp[j,e]  = sum_d vT[d,j] * wvT[d,e]      (256 MMs)
    outT[e,i]= (sum_j vp[j,e] * eT[j,i]) * (1/cs[i])   (256 MMs)

Matmuls run as float32r (fp32 bytes, single-pass PE mode, ~4x the
fp32 rate). All matmul operands are produced directly in float32r
(DMA loads and engine writes), satisfying the BIR verifier's
"rounded to FP32r" rule. Measured end-to-end relative error vs the
fp32 reference: ~4e-4.

SBUF budget is ~208KB/partition, managed as two allocation stacks
(left/right) with phase-scoped pools. Inputs stream through small
rotating chunk pools ([128,512] tiles, 2 slots per contraction tile)
in first-use order so DMA overlaps compute; 52 warm-up matmuls on a
constant tile keep the PE HAM clock at full rate while the first
input chunks land. Colsum matmuls trail their exp by one group so
the in-order PE never waits on the ACT engine.

exp(sT) round-trips through DRAM (staged exp tiles DMA out during
the score phase, streamed back in i-slice halves with a split-j
accumulation in the output phase). That frees 64KB of SBUF, which
lets wv prefetch during earlier phases via the weight-pool rotation
— the PE runs gap-free from warm-up to the last matmul and the HAM
clock stays at 2.4GHz for the whole kernel.

Measured on 8 axon-attached TRN2 cores: ~304 us HW exec time
(PE-limited; 1184 N=512 fp32r matmuls/core stream at ~233 ns each;
phases A-F all within ~2% of the matmul issue-rate floor).
"""

import numpy as np

_B, _S, _DK = 4, 2048, 1024
_HALF = _S // 2
_N_CORES = 8
_P = 128

_CACHE = {}


def _emit(tc, qT, kT, vT, wqT, wkT, wvT, outT, DK, S, HALF, mm_dt):
    import concourse.bass as bass
    from concourse import mybir

    nc = tc.nc
    ts = bass.ts
    P = _P
    NF = min(512, HALF, S, DK)
    DT = DK // P        # contraction tiles (d)
    ET = DK // P        # output-feature tiles (e)
    JT = S // P         # key tiles (j)
    ISL = HALF // NF    # query slices (i)
    JSL = S // NF       # key slices
    ESL = DK // NF      # feature slices
    JGN = S // NF       # vT chunk groups (NF//P j-tiles each)
    JPG = NF // P       # j-tiles per vT chunk
    NORM = 1.0 / float(np.sqrt(DK))
    f32 = mybir.dt.float32
    AF = mybir.ActivationFunctionType

    _cms = {}

    def opn(**kw):
        cm = tc.tile_pool(**kw)
        pool = cm.__enter__()
        _cms[id(pool)] = cm
        return pool

    def cls(*pools):
        for pool in pools:
            _cms.pop(id(pool)).__exit__(None, None, None)

    # ---------------- pools ----------------
    # LEFT stack: misc | x (stream rotation) | kpT | qpT | later vp, wv
    # RIGHT stack: stage | w (wk/wq chunks) | later eT
    misc = opn(name="misc", bufs=1, side="left")
    xp = opn(name="xp", bufs=1, side="left")
    stage = opn(name="stage", bufs=2, side="right")
    wp = opn(name="wp", bufs=1, side="right")
    psmm = opn(name="psmm", bufs=6, space="PSUM")
    psacc = opn(name="psacc", bufs=1, space="PSUM")
    dram = opn(name="dram", bufs=1, space="DRAM")
    eT_dram = dram.tile([S, HALF], mm_dt, name="et_dram")

    ones_f32 = misc.tile([P, P], f32, tag="ones_f32")
    nc.vector.memset(ones_f32[:], 1.0)
    ones = misc.tile([P, P], mm_dt, tag="ones")
    nc.vector.tensor_copy(ones[:], ones_f32[:])
    recip = misc.tile([P, HALF], f32, tag="recip")
    cs_ps = [psacc.tile([P, NF], f32, tag=f"cs{i}", name=f"cs{i}") for i in range(ISL)]

    # x-pool rotation: per-d stream chunks [P, NF], 2 slots.
    # Allocation order per d: k[0..JSL-1], q[0..ISL-1], vs[0..JGN-1].
    def x_tile(kind, d, idx):
        return xp.tile([P, NF], mm_dt, tag=f"x{d}", bufs=2, name=f"{kind}{idx}_d{d}")

    # ---------------- PE warm-up while first DMAs land ----------------
    warm_ps = psmm.tile([P, P], f32, tag="mm", name="warm_ps")
    for _ in range(52):
        nc.tensor.matmul(warm_ps[:], ones[:], ones[:], start=True, stop=True)

    # ---------------- phase A: kpT = (k @ wk.T).T ----------------
    kp_pool = opn(name="kpp", bufs=1, side="left")
    kpT = [kp_pool.tile([P, S], mm_dt, tag=f"kp{e}", name=f"kp{e}") for e in range(ET)]

    # wk/wq chunk slots [P, NF] (e-halves), 2 bufs: slot0 = wk, slot1 = wq
    EPC = NF // P  # e-tiles per w chunk
    WH = ET // EPC  # w chunks per d
    wk_c = [[None] * WH for _ in range(DT)]
    wq_c = [[None] * WH for _ in range(DT)]

    def load_w(dst, d, h, src, nm, eng=None):
        t = wp.tile([P, NF], mm_dt, tag=f"w{d}h{h}", bufs=2, name=f"{nm}{d}_{h}")
        (eng or nc.sync).dma_start(t[:], src[ts(d, P), ts(h, NF)])
        dst[d][h] = t

    def w_slice(c, d, e):
        return c[d][e // EPC][:, ts(e % EPC, P)]

    k_c = [[None] * JSL for _ in range(DT)]
    q_c = [[None] * ISL for _ in range(DT)]
    vs_c = [[None] * JGN for _ in range(DT)]

    # first-use-ordered input streaming: wk[*][h0], k[*][js0], wk[*][h1..]
    for d in range(DT):
        load_w(wk_c, d, 0, wkT, "wk")
    for d in range(DT):
        k_c[d][0] = x_tile("k", d, 0)
        nc.sync.dma_start(k_c[d][0][:], kT[ts(d, P), ts(0, NF)])
    for h in range(1, WH):
        for d in range(DT):
            load_w(wk_c, d, h, wkT, "wk")

    for js in range(JSL):
        if js + 1 < JSL:  # prefetch next k slab
            for d in range(DT):
                k_c[d][js + 1] = x_tile("k", d, js + 1)
                nc.sync.dma_start(k_c[d][js + 1][:], kT[ts(d, P), ts(js + 1, NF)])
        if js == min(1, JSL - 1):  # wq loads ride behind early k prefetches
            for h in range(WH):
                for d in range(DT):
                    load_w(wq_c, d, h, wqT, "wq")
        if js == min(2, JSL - 1):  # q[isl0] into freed k slots
            for d in range(DT):
                q_c[d][0] = x_tile("q", d, 0)
                nc.sync.dma_start(q_c[d][0][:], qT[ts(d, P), ts(0, NF)])
        if js == JSL - 1:  # remaining q slices
            for isl in range(1, ISL):
                for d in range(DT):
                    q_c[d][isl] = x_tile("q", d, isl)
                    nc.sync.dma_start(q_c[d][isl][:], qT[ts(d, P), ts(isl, NF)])
        for e in range(ET):
            ps = psmm.tile([P, NF], f32, tag="mm")
            for d in range(DT):
                nc.tensor.matmul(
                    ps[:],
                    w_slice(wk_c, d, e),
                    k_c[d][js][:],
                    start=(d == 0),
                    stop=(d == DT - 1),
                )
            nc.vector.tensor_copy(kpT[e][:, ts(js, NF)], ps[:])

    # ---------------- phase B: qpT = (q @ wq.T).T ----------------
    qp_pool = opn(name="qpp", bufs=1, side="left")
    qpT = [
        qp_pool.tile([P, HALF], mm_dt, tag=f"qp{e}", name=f"qp{e}") for e in range(ET)
    ]
    for isl in range(ISL):
        for e in range(ET):
            ps = psmm.tile([P, NF], f32, tag="mm")
            for d in range(DT):
                nc.tensor.matmul(
                    ps[:],
                    w_slice(wq_c, d, e),
                    q_c[d][isl][:],
                    start=(d == 0),
                    stop=(d == DT - 1),
                )
            nc.vector.tensor_copy(qpT[e][:, ts(isl, NF)], ps[:])
    # wv rides the w-rotation (slot freed when wk releases at end of A),
    # so it loads during B/C — no stall at the C->E boundary.
    wv_c = [[None] * WH for _ in range(DT)]
    for h in range(WH):
        for d in range(DT):
            load_w(wv_c, d, h, wvT, "wv")

    # ---------------- phase C: sT -> exp -> eT_dram (+ colsum), vs prefetch ----
    etsp = opn(name="etsp", bufs=1, side="right")
    # vs chunks 0/1 drain as soon as q slots free (mid/end of phase B)
    for g in range(min(2, JGN)):
        for d in range(DT):
            vs_c[d][g] = x_tile("vs", d, g)
            nc.sync.dma_start(vs_c[d][g][:], vT[ts(d, P), ts(g, NF)])
    pending_cs = []
    for j in range(JT):
        for isl in range(ISL):
            ps = psmm.tile([P, NF], f32, tag="mm")
            for e in range(ET):
                nc.tensor.matmul(
                    ps[:],
                    kpT[e][:, ts(j, P)],
                    qpT[e][:, ts(isl, NF)],
                    start=(e == 0),
                    stop=(e == ET - 1),
                )
            st = etsp.tile([P, NF], mm_dt, tag="ets", bufs=3, name=f"ets{j}_{isl}")
            nc.scalar.activation(st[:], ps[:], AF.Exp, scale=NORM)
            nc.sync.dma_start(eT_dram[ts(j, P), ts(isl, NF)], st[:])
            pending_cs.append((j, isl, st))
            if len(pending_cs) > 1:
                pj, pisl, pst = pending_cs.pop(0)
                nc.tensor.matmul(
                    cs_ps[pisl][:],
                    ones[:],
                    pst[:],
                    start=(pj == 0),
                    stop=(pj == JT - 1),
                )
    for pj, pisl, pst in pending_cs:
        nc.tensor.matmul(
            cs_ps[pisl][:],
            ones[:],
            pst[:],
            start=(pj == 0),
            stop=(pj == JT - 1),
        )
    for isl in range(ISL):
        nc.vector.reciprocal(recip[:, ts(isl, NF)], cs_ps[isl][:])
    cls(etsp)
    cls(qp_pool, kp_pool)
    cls(psacc)

    # ---------------- phase E: vp = v @ wv.T ----------------
    vp_pool = opn(name="vpp", bufs=1, side="left")
    vp = [vp_pool.tile([P, DK], mm_dt, tag=f"vp{j}", name=f"vp{j}") for j in range(JT)]
    # eT comes back from DRAM in halves during E/F (tag rotation per j%8)
    ethp = opn(name="ethp", bufs=1, side="left")
    eth = [[None] * JT for _ in range(ISL)]

    def load_eth(isl, jlist):
        for j in jlist:
            t = ethp.tile(
                [P, NF], mm_dt, tag=f"eh{j % 8}", bufs=2, name=f"eh{isl}_{j}"
            )
            nc.sync.dma_start(t[:], eT_dram[ts(j, P), ts(isl, NF)])
            eth[isl][j] = t

    load_eth(0, range(JT // 2))
    load_eth(0, range(JT // 2, JT))
    for g in range(JGN):
        if g + 2 < JGN:  # double-buffered vs prefetch
            gg = g + 2
            for d in range(DT):
                vs_c[d][gg] = x_tile("vs", d, gg)
                nc.sync.dma_start(vs_c[d][gg][:], vT[ts(d, P), ts(gg, NF)])
        for jin in range(JPG):
            j = g * JPG + jin
            for es in range(ESL):
                ps = psmm.tile([P, NF], f32, tag="mm")
                for d in range(DT):
                    nc.tensor.matmul(
                        ps[:],
                        vs_c[d][g][:, ts(jin, P)],
                        wv_c[d][es][:],
                        start=(d == 0),
                        stop=(d == DT - 1),
                    )
                nc.vector.tensor_copy(vp[j][:, ts(es, NF)], ps[:])
    cls(wp)
    cls(psmm)

    # ---------------- phase F: outT = (eT.T @ vp).T * recip ----------------
    # Two j-half passes per i-slice so eth tiles release mid-slice and the
    # next slice's eth loads prefetch without a stall. One PSUM bank per e.
    pf = opn(name="pf", bufs=1, space="PSUM")
    JH = JT // 2
    for isl in range(ISL):
        pft = [
            pf.tile([P, NF], f32, tag=f"pf{e}", name=f"pf{e}_{isl}")
            for e in range(ET)
        ]
        for e in range(ET):
            for j in range(JH):
                nc.tensor.matmul(
                    pft[e][:],
                    vp[j][:, ts(e, P)],
                    eth[isl][j][:],
                    start=(j == 0),
                    stop=False,
                )
        if isl + 1 < ISL:
            load_eth(isl + 1, range(JH))
        for e in range(ET):
            for j in range(JH, JT):
                nc.tensor.matmul(
                    pft[e][:],
                    vp[j][:, ts(e, P)],
                    eth[isl][j][:],
                    start=False,
                    stop=(j == JT - 1),
                )
            ot = stage.tile([P, NF], f32, tag="ost")
            nc.vector.tensor_mul(ot[:], pft[e][:], recip[:, ts(isl, NF)])
            nc.sync.dma_start(outT[ts(e, P), ts(isl, NF)], ot[:])
        if isl + 1 < ISL:
            load_eth(isl + 1, range(JH, JT))
    cls(ethp, vp_pool, xp, misc, stage, pf, dram)


def build_program(DK=_DK, S=_S, HALF=_HALF, mm_dtype="bfloat16"):
    """Build + compile the per-core Bass program. Returns the Bacc object."""
    import concourse.tile as tile
    from concourse import bacc, mybir

    f32 = mybir.dt.float32
    mm_dt = getattr(mybir.dt, mm_dtype)

    nc = bacc.Bacc(
        "TRN2",
        target_bir_lowering=False,
        debug=False,
        enable_asserts=False,
        num_devices=_N_CORES,
    )
    qT = nc.dram_tensor("qt", (DK, HALF), mm_dt, kind="ExternalInput").ap()
    kT = nc.dram_tensor("kt", (DK, S), mm_dt, kind="ExternalInput").ap()
    vT = nc.dram_tensor("vt", (DK, S), mm_dt, kind="ExternalInput").ap()
    wqT = nc.dram_tensor("wqt", (DK, DK), mm_dt, kind="ExternalInput").ap()
    wkT = nc.dram_tensor("wkt", (DK, DK), mm_dt, kind="ExternalInput").ap()
    wvT = nc.dram_tensor("wvt", (DK, DK), mm_dt, kind="ExternalInput").ap()
    outT = nc.dram_tensor("outt", (DK, HALF), f32, kind="ExternalOutput").ap()

    with tile.TileContext(nc) as tc:
        _emit(tc, qT, kT, vT, wqT, wkT, wvT, outT, DK, S, HALF, mm_dt)
    nc.compile()
    return nc


def _in_maps(q, k, v, wq, wk, wv):
    """Shard full inputs into 8 per-core input maps (host-side transposes)."""
    import ml_dtypes

    bf16 = ml_dtypes.bfloat16
    wqT = np.ascontiguousarray(wq.T).astype(bf16)
    wkT = np.ascontiguousarray(wk.T).astype(bf16)
    wvT = np.ascontiguousarray(wv.T).astype(bf16)
    kT_b = [np.ascontiguousarray(k[b].T).astype(bf16) for b in range(_B)]
    vT_b = [np.ascontiguousarray(v[b].T).astype(bf16) for b in range(_B)]
    maps = []
    for c in range(_N_CORES):
        b, h = divmod(c, 2)
        qT = np.ascontiguousarray(q[b, h * _HALF : (h + 1) * _HALF, :].T).astype(bf16)
        maps.append(
            {
                "qt": qT,
                "kt": kT_b[b],
                "vt": vT_b[b],
                "wqt": wqT,
                "wkt": wkT,
                "wvt": wvT,
            }
        )
    return maps


def kernel(q, k, v, wq, wk, wv):
    from concourse.bass_utils import run_bass_kernel_spmd

    q = np.asarray(q, np.float32)
    k = np.asarray(k, np.float32)
    v = np.asarray(v, np.float32)
    wq = np.asarray(wq, np.float32)
    wk = np.asarray(wk, np.float32)
    wv = np.asarray(wv, np.float32)

    if "nc" not in _CACHE:
        _CACHE["nc"] = build_program()
    nc = _CACHE["nc"]

    res = run_bass_kernel_spmd(
        nc, _in_maps(q, k, v, wq, wk, wv), core_ids=list(range(_N_CORES))
    )

    out = np.empty((_B, _S, _DK), np.float32)
    for c in range(_N_CORES):
        b, h = divmod(c, 2)
        out[b, h * _HALF : (h + 1) * _HALF, :] = res.results[c]["outt"].T
    return out



# revision 10
# speedup vs baseline: 1.1065x; 1.0163x over previous
"""Trainium2 Bass kernel for nn_AttentionHead (B=4, S=2048, DK=1024).

Single-head attention with input projections:
    qp = q @ wq.T; kp = k @ wk.T; vp = v @ wv.T
    s  = qp @ kp.T / sqrt(dk); attn = softmax(s); out = attn @ vp

Sharding: 8 cores = (batch b in 0..3) x (sequence half h in 0..1).
Each core owns 1024 query rows AND 1024 key/value rows of its batch.
K/V projections are computed once per row (no duplication across the
pair): each core projects only its own 1024 k/v rows, then the pair
exchanges halves with a 2-rank AllGather through HBM bounce buffers
(SPMD-uniform layout: both halves are read back from the AllGather
output in global j-order, so the program is identical on all cores).

Per core (all matmul operands bf16, fp32 PSUM accumulation):
    A: kpT_loc[e,j'] = sum_d wkT[d,e] kT_loc[d,j']   (128 MMs)
       -> bounce -> AllGather(pair) -> kpT[e, 0:2048]
    E: vp_loc[j',e]  = sum_d vT_loc[d,j'] wvT[d,e]   (128 MMs)
       -> bounce -> AllGather(pair) -> vp[j, 0:1024] for all 16 j-tiles
    B: qpT[e,i]  = sum_d wqT[d,e] qT[d,i]            (128 MMs)
    C: sT[j,i]   = sum_e kpT[e,j] qpT[e,i]           (256 MMs)
       eT[j,i]   = exp(sT/32)  (ACT, fused scale, stays in SBUF)
       cs[i]     = sum_j eT[j,i] via ones-matmul      (32 MMs, trailing)
    F: outT[e,i] = (sum_j vp[j,e] eT[j,i]) * (1/cs[i])  (256 MMs)

928 matmuls/core at ~216 ns each (bf16 streams 1 col/cycle like fp32r
but FWL halves the LDWEIGHTS bubble; measured vs 233.8 ns for fp32r).
Collectives (4 x 1MB pair-AllGathers) are kicked as soon as each half
of kp/vp is produced and consumed 2-3 phases later, so they hide
behind compute. Measured end-to-end relative error vs the fp32
reference: ~6e-3 (bf16 quantization of inputs + intermediates).

SBUF: ~173KB/partition peak (two allocation stacks). Inputs stream
through rotating [128,512] chunk pools in first-use order (k, v, q)
so DMA overlaps compute; 52 warm-up matmuls keep the PE HAM clock
ramped while the first input chunks land.
"""

import numpy as np

_B, _S, _DK = 4, 2048, 1024
_HALF = _S // 2
_N_CORES = 8
_P = 128
_PAIRS = [[0, 1], [2, 3], [4, 5], [6, 7]]

_CACHE = {}


def _emit(tc, qT, kTh, vTh, wqT, wkT, wvT, outT, cc, DK, S, HALF, mm_dt):
    import concourse.bass as bass
    from concourse import mybir

    nc = tc.nc
    ts = bass.ts
    P = _P
    NF = 512
    KH = S // 2            # local key/value rows
    DT = DK // P           # contraction tiles (d)
    ET = DK // P           # output-feature tiles (e)
    JT = S // P            # global key tiles (j)
    ISL = HALF // NF       # query slices (i)
    JSLH = KH // NF        # local key slices
    ESL = DK // NF         # feature slices
    JGN = KH // NF         # local vT chunk groups
    JPG = NF // P          # j-tiles per vT chunk
    NORM = 1.0 / float(np.sqrt(DK))
    f32 = mybir.dt.float32
    AF = mybir.ActivationFunctionType
    kp_in, kp_out, vp_in, vp_out = cc

    _cms = {}

    def opn(**kw):
        cm = tc.tile_pool(**kw)
        pool = cm.__enter__()
        _cms[id(pool)] = cm
        return pool

    def cls(*pools):
        for pool in pools:
            _cms.pop(id(pool)).__exit__(None, None, None)

    # ---------------- pools ----------------
    # LEFT stack: misc | x (stream rotation) | kpT | qpT | vp
    # RIGHT stack: stage | cst (bounce staging) | w chunks | eT
    misc = opn(name="misc", bufs=1, side="left")
    xp = opn(name="xp", bufs=1, side="left")
    stage = opn(name="stage", bufs=2, side="right")
    cstp = opn(name="cstp", bufs=1, side="right")
    wp = opn(name="wp", bufs=1, side="right")
    psmm = opn(name="psmm", bufs=6, space="PSUM")
    psacc = opn(name="psacc", bufs=1, space="PSUM")

    ones_f32 = misc.tile([P, P], f32, tag="ones_f32")
    nc.vector.memset(ones_f32[:], 1.0)
    ones = misc.tile([P, P], mm_dt, tag="ones")
    nc.vector.tensor_copy(ones[:], ones_f32[:])
    recip = misc.tile([P, HALF], f32, tag="recip")
    cs_ps = [psacc.tile([P, NF], f32, tag=f"cs{i}", name=f"cs{i}") for i in range(ISL)]

    # x-pool rotation: per-d stream chunks [P, NF], 2 slots.
    # Allocation order per d: k[0..1], vs[0..1], q[0..1].
    def x_tile(kind, d, idx):
        return xp.tile([P, NF], mm_dt, tag=f"x{d}", bufs=2, name=f"{kind}{idx}_d{d}")

    def cst_tile(nm):
        return cstp.tile([P, NF], mm_dt, tag="cst", bufs=4, name=nm)

    # ---------------- PE warm-up while first DMAs land ----------------
    warm_ps = psmm.tile([P, P], f32, tag="mm", name="warm_ps")
    for _ in range(52):
        nc.tensor.matmul(warm_ps[:], ones[:], ones[:], start=True, stop=True)

    # weight chunk slots [P, NF] (e-halves), 2 bufs: rotation wk -> wv -> wq
    EPC = NF // P   # e-tiles per w chunk
    WH = ET // EPC  # w chunks per d
    wk_c = [[None] * WH for _ in range(DT)]
    wv_c = [[None] * WH for _ in range(DT)]
    wq_c = [[None] * WH for _ in range(DT)]

    def load_w(dst, d, h, src, nm):
        t = wp.tile([P, NF], mm_dt, tag=f"w{d}h{h}", bufs=2, name=f"{nm}{d}_{h}")
        nc.sync.dma_start(t[:], src[ts(d, P), ts(h, NF)])
        dst[d][h] = t

    def w_slice(c, d, e):
        return c[d][e // EPC][:, ts(e % EPC, P)]

    k_c = [[None] * JSLH for _ in range(DT)]
    vs_c = [[None] * JGN for _ in range(DT)]
    q_c = [[None] * ISL for _ in range(DT)]

    # first-use-ordered input streaming: wk[*][h0], k[*][js0], wk[*][h1]
    for d in range(DT):
        load_w(wk_c, d, 0, wkT, "wk")
    for d in range(DT):
        k_c[d][0] = x_tile("k", d, 0)
        nc.sync.dma_start(k_c[d][0][:], kTh[ts(d, P), ts(0, NF)])
    for h in range(1, WH):
        for d in range(DT):
            load_w(wk_c, d, h, wkT, "wk")

    # ---------------- phase A: local kpT half -> bounce -> AllGather ----
    kp_pool = opn(name="kpp", bufs=1, side="left")
    kpT = [kp_pool.tile([P, S], mm_dt, tag=f"kp{e}", name=f"kp{e}") for e in range(ET)]

    for js in range(JSLH):
        if js + 1 < JSLH:  # prefetch next k slab
            for d in range(DT):
                k_c[d][js + 1] = x_tile("k", d, js + 1)
                nc.sync.dma_start(k_c[d][js + 1][:], kTh[ts(d, P), ts(js + 1, NF)])
        if js == 0:  # wv rides behind early prefetches (first use: phase E)
            for h in range(WH):
                for d in range(DT):
                    load_w(wv_c, d, h, wvT, "wv")
        for e in range(ET):
            ps = psmm.tile([P, NF], f32, tag="mm")
            for d in range(DT):
                nc.tensor.matmul(
                    ps[:],
                    w_slice(wk_c, d, e),
                    k_c[d][js][:],
                    start=(d == 0),
                    stop=(d == DT - 1),
                )
            st = cst_tile(f"kpb{js}_{e}")
            nc.vector.tensor_copy(st[:], ps[:])
            nc.sync.dma_start(kp_in[js][ts(e, P), :], st[:])
        # vs chunk js drains into freed k slot while the collective runs
        for d in range(DT):
            vs_c[d][js] = x_tile("vs", d, js)
            nc.sync.dma_start(vs_c[d][js][:], vTh[ts(d, P), ts(js, NF)])
        nc.gpsimd.collective_compute(
            "AllGather",
            mybir.AluOpType.bypass,
            replica_groups=_PAIRS,
            ins=[kp_in[js][:, :]],
            outs=[kp_out[js][:, :]],
        )

    # ---------------- phase E: local vp half -> bounce -> AllGather ----
    vp_pool = opn(name="vpp", bufs=1, side="left")
    vp = [vp_pool.tile([P, DK], mm_dt, tag=f"vp{j}", name=f"vp{j}") for j in range(JT)]

    for g in range(JGN):
        for jin in range(JPG):
            for es in range(ESL):
                ps = psmm.tile([P, NF], f32, tag="mm")
                for d in range(DT):
                    nc.tensor.matmul(
                        ps[:],
                        vs_c[d][g][:, ts(jin, P)],
                        wv_c[d][es][:],
                        start=(d == 0),
                        stop=(d == DT - 1),
                    )
                st = cst_tile(f"vpb{g}_{jin}_{es}")
                nc.vector.tensor_copy(st[:], ps[:])
                nc.sync.dma_start(
                    vp_in[g][ts(jin, P), ts(es, NF)], st[:]
                )
        # q slice g drains into freed vs slot while the collective runs
        for d in range(DT):
            q_c[d][g] = x_tile("q", d, g)
            nc.sync.dma_start(q_c[d][g][:], qT[ts(d, P), ts(g, NF)])
        if g == 0:  # wq rides the w-rotation (wk slots freed at end of A)
            for h in range(WH):
                for d in range(DT):
                    load_w(wq_c, d, h, wqT, "wq")
        nc.gpsimd.collective_compute(
            "AllGather",
            mybir.AluOpType.bypass,
            replica_groups=_PAIRS,
            ins=[vp_in[g][:, :]],
            outs=[vp_out[g][:, :]],
        )

    # kp readback, issued only after ALL input-stream DMAs are queued:
    # a readback descriptor waits on its collective and would head-of-line
    # block any input load queued behind it. Both halves are read back in
    # global j-order so tile placement is core-independent (SPMD-uniform).
    for js in range(JSLH):
        for hr in range(2):
            for e in range(ET):
                nc.sync.dma_start(
                    kpT[e][:, ts(hr * JSLH + js, NF)],
                    kp_out[js][ts(hr * ET + e, P), :],
                )

    # ---------------- phase B: qpT = (q @ wq.T).T ----------------
    qp_pool = opn(name="qpp", bufs=1, side="left")
    qpT = [
        qp_pool.tile([P, HALF], mm_dt, tag=f"qp{e}", name=f"qp{e}") for e in range(ET)
    ]
    for isl in range(ISL):
        for e in range(ET):
            ps = psmm.tile([P, NF], f32, tag="mm")
            for d in range(DT):
                nc.tensor.matmul(
                    ps[:],
                    w_slice(wq_c, d, e),
                    q_c[d][isl][:],
                    start=(d == 0),
                    stop=(d == DT - 1),
                )
            nc.vector.tensor_copy(qpT[e][:, ts(isl, NF)], ps[:])

    # vp readback (collective-gated; nothing but phase-F output staging is
    # queued behind these, and the vp collectives complete during B/C)
    for g in range(JGN):
        for hr in range(2):
            for jin in range(JPG):
                j = hr * (JT // 2) + g * JPG + jin
                nc.sync.dma_start(vp[j][:, :], vp_out[g][ts(hr * JPG + jin, P), :])

    # ---------------- phase C: sT -> exp -> eT (SBUF) + trailing colsum ----
    et_pool = opn(name="etp", bufs=1, side="right")
    eT = [et_pool.tile([P, HALF], mm_dt, tag=f"et{j}", name=f"et{j}") for j in range(JT)]
    pending_cs = []
    for j in range(JT):
        for isl in range(ISL):
            ps = psmm.tile([P, NF], f32, tag="mm")
            for e in range(ET):
                nc.tensor.matmul(
                    ps[:],
                    kpT[e][:, ts(j, P)],
                    qpT[e][:, ts(isl, NF)],
                    start=(e == 0),
                    stop=(e == ET - 1),
                )
            nc.scalar.activation(eT[j][:, ts(isl, NF)], ps[:], AF.Exp, scale=NORM)
            pending_cs.append((j, isl))
            if len(pending_cs) > 1:
                pj, pisl = pending_cs.pop(0)
                nc.tensor.matmul(
                    cs_ps[pisl][:],
                    ones[:],
                    eT[pj][:, ts(pisl, NF)],
                    start=(pj == 0),
                    stop=(pj == JT - 1),
                )
    for pj, pisl in pending_cs:
        nc.tensor.matmul(
            cs_ps[pisl][:],
            ones[:],
            eT[pj][:, ts(pisl, NF)],
            start=(pj == 0),
            stop=(pj == JT - 1),
        )
    for isl in range(ISL):
        nc.vector.reciprocal(recip[:, ts(isl, NF)], cs_ps[isl][:])
    cls(psacc)
    cls(psmm)

    # ---------------- phase F: outT = (eT.T @ vp).T * recip ----------------
    pf = opn(name="pf", bufs=1, space="PSUM")
    for isl in range(ISL):
        for e in range(ET):
            pft = pf.tile([P, NF], f32, tag=f"pf{e}", name=f"pf{e}_{isl}")
            for j in range(JT):
                nc.tensor.matmul(
                    pft[:],
                    vp[j][:, ts(e, P)],
                    eT[j][:, ts(isl, NF)],
                    start=(j == 0),
                    stop=(j == JT - 1),
                )
            ot = stage.tile([P, NF], f32, tag="ost")
            nc.vector.tensor_mul(ot[:], pft[:], recip[:, ts(isl, NF)])
            nc.sync.dma_start(outT[ts(e, P), ts(isl, NF)], ot[:])
    cls(qp_pool, vp_pool, kp_pool, xp, misc)
    cls(et_pool, wp, cstp, stage)
    cls(pf)


def build_program(DK=_DK, S=_S, HALF=_HALF, mm_dtype="bfloat16"):
    """Build + compile the per-core Bass program. Returns the Bacc object."""
    import concourse.tile as tile
    from concourse import bacc, mybir

    f32 = mybir.dt.float32
    mm_dt = getattr(mybir.dt, mm_dtype)
    KH = S // 2
    NF = 512

    nc = bacc.Bacc(
        "TRN2",
        target_bir_lowering=False,
        debug=False,
        enable_asserts=False,
        num_devices=_N_CORES,
    )
    qT = nc.dram_tensor("qt", (DK, HALF), mm_dt, kind="ExternalInput").ap()
    kTh = nc.dram_tensor("kt", (DK, KH), mm_dt, kind="ExternalInput").ap()
    vTh = nc.dram_tensor("vt", (DK, KH), mm_dt, kind="ExternalInput").ap()
    wqT = nc.dram_tensor("wqt", (DK, DK), mm_dt, kind="ExternalInput").ap()
    wkT = nc.dram_tensor("wkt", (DK, DK), mm_dt, kind="ExternalInput").ap()
    wvT = nc.dram_tensor("wvt", (DK, DK), mm_dt, kind="ExternalInput").ap()
    outT = nc.dram_tensor("outt", (DK, HALF), f32, kind="ExternalOutput").ap()

    # HBM bounce buffers for the pair AllGathers (2 chunks per tensor so
    # each collective kicks off as soon as its half-slab is produced)
    kp_in = [
        nc.dram_tensor(f"kp_in{c}", (DK, NF), mm_dt, kind="Internal").ap()
        for c in range(KH // NF)
    ]
    kp_out = [
        nc.dram_tensor(f"kp_out{c}", (2 * DK, NF), mm_dt, kind="Internal").ap()
        for c in range(KH // NF)
    ]
    vp_in = [
        nc.dram_tensor(f"vp_in{g}", (KH // 2, DK), mm_dt, kind="Internal").ap()
        for g in range(KH // NF)
    ]
    vp_out = [
        nc.dram_tensor(f"vp_out{g}", (KH, DK), mm_dt, kind="Internal").ap()
        for g in range(KH // NF)
    ]

    with tile.TileContext(nc) as tc:
        _emit(
            tc,
            qT,
            kTh,
            vTh,
            wqT,
            wkT,
            wvT,
            outT,
            (kp_in, kp_out, vp_in, vp_out),
            DK,
            S,
            HALF,
            mm_dt,
        )
    nc.compile()
    return nc


def _in_maps(q, k, v, wq, wk, wv):
    """Shard full inputs into 8 per-core input maps (host-side transposes)."""
    import ml_dtypes

    bf16 = ml_dtypes.bfloat16
    wqT = np.ascontiguousarray(wq.T).astype(bf16)
    wkT = np.ascontiguousarray(wk.T).astype(bf16)
    wvT = np.ascontiguousarray(wv.T).astype(bf16)
    maps = []
    for c in range(_N_CORES):
        b, h = divmod(c, 2)
        sl = slice(h * _HALF, (h + 1) * _HALF)
        maps.append(
            {
                "qt": np.ascontiguousarray(q[b, sl, :].T).astype(bf16),
                "kt": np.ascontiguousarray(k[b, sl, :].T).astype(bf16),
                "vt": np.ascontiguousarray(v[b, sl, :].T).astype(bf16),
                "wqt": wqT,
                "wkt": wkT,
                "wvt": wvT,
            }
        )
    return maps


def kernel(q, k, v, wq, wk, wv):
    from concourse.bass_utils import run_bass_kernel_spmd

    q = np.asarray(q, np.float32)
    k = np.asarray(k, np.float32)
    v = np.asarray(v, np.float32)
    wq = np.asarray(wq, np.float32)
    wk = np.asarray(wk, np.float32)
    wv = np.asarray(wv, np.float32)

    if "nc" not in _CACHE:
        _CACHE["nc"] = build_program()
    nc = _CACHE["nc"]

    res = run_bass_kernel_spmd(
        nc, _in_maps(q, k, v, wq, wk, wv), core_ids=list(range(_N_CORES))
    )

    out = np.empty((_B, _S, _DK), np.float32)
    for c in range(_N_CORES):
        b, h = divmod(c, 2)
        out[b, h * _HALF : (h + 1) * _HALF, :] = res.results[c]["outt"].T
    return out


# revision 13
# speedup vs baseline: 1.1171x; 1.0096x over previous
"""Trainium2 Bass kernel for nn_AttentionHead (B=4, S=2048, DK=1024).

Single-head attention with input projections:
    qp = q @ wq.T; kp = k @ wk.T; vp = v @ wv.T
    s  = qp @ kp.T / sqrt(dk); attn = softmax(s); out = attn @ vp

Sharding: 8 cores = (batch b in 0..3) x (sequence half h in 0..1).
Each core owns 1024 query rows AND 1024 key/value rows of its batch.
K/V projections are computed once per row (no duplication across the
pair): each core projects only its own 1024 k/v rows, then the pair
exchanges halves with 2-rank AllGathers through HBM bounce buffers
(SPMD-uniform layout: both halves are read back from the AllGather
output in global j-order, so the program is identical on all cores).

Per core (all matmul operands bf16, fp32 PSUM accumulation):
    A: kpT_loc[e,j'] = sum_d wkT[d,e] kT_loc[d,j']   (128 MMs)
       -> bounce -> AllGather(pair) -> kpT[e, 0:2048]
    E: vp_loc[j',e]  = sum_d vT_loc[d,j'] wvT[d,e]   (128 MMs)
       -> bounce -> AllGather(pair) -> vp[j] for all 16 j-tiles
    B: qpT[e,i]  = sum_d wqT[d,e] qT[d,i]            (128 MMs)
    C: sT[j,i]   = sum_e kpT[e,j] qpT[e,i]           (256 MMs)
       eT[j,i]   = exp(sT/32)  (ACT, fused scale, stays in SBUF)
       cs[i]     = sum_j eT[j,i] via ones-matmul      (32 MMs, trailing)
    F: outT[e,i] = (sum_j vp[j,e] eT[j,i]) * (1/cs[i])  (256 MMs)

DMA-descriptor generation is the hidden serial resource: each
dma_start costs ~0.6us of descriptor generation on its issuing
engine's sequencer, strictly in program order. The ~200 DMAs are
split across BOTH hardware DGE rings so neither backs up: input
streams + AllGather readbacks on the sync ring (consumption order),
bounce writes + output stores on the scalar ring. Collectives
trigger from gpsimd. Each ring's program order equals it
consumption order, so no head-of-line inversion is possible.

Phase-F accumulators come from the same rotating 6-bank PSUM pool
as the earlier phases (no pool close/open at the C->F boundary --
a fresh pool would wait on the colsum-reciprocal chain and stall
the PE ~6us). The reciprocal runs on the scalar engine.

928 matmuls/core at ~216 ns each (bf16; FWL + LDWEIGHTS prefetch
hide the weight-load bubble). Measured end-to-end relative error vs
the fp32 reference: ~6e-3 (bf16 quantization).
"""

import numpy as np

_B, _S, _DK = 4, 2048, 1024
_HALF = _S // 2
_N_CORES = 8
_P = 128
_PAIRS = [[0, 1], [2, 3], [4, 5], [6, 7]]

_CACHE = {}


def _emit(tc, qT, kTh, vTh, wqT, wkT, wvT, outT, cc, DK, S, HALF, mm_dt):
    import concourse.bass as bass
    from concourse import mybir

    nc = tc.nc
    ts = bass.ts
    P = _P
    NF = 512
    KH = S // 2            # local key/value rows
    DT = DK // P           # contraction tiles (d)
    ET = DK // P           # output-feature tiles (e)
    JT = S // P            # global key tiles (j)
    ISL = HALF // NF       # query slices (i)
    JSLH = KH // NF        # local key slices
    ESL = DK // NF         # feature slices
    JGN = KH // NF         # local vT chunk groups
    JPG = NF // P          # j-tiles per vT chunk
    NORM = 1.0 / float(np.sqrt(DK))
    f32 = mybir.dt.float32
    AF = mybir.ActivationFunctionType
    kp_in, kp_out, vp_in, vp_out = cc

    _cms = {}

    def opn(**kw):
        cm = tc.tile_pool(**kw)
        pool = cm.__enter__()
        _cms[id(pool)] = cm
        return pool

    def cls(*pools):
        for pool in pools:
            _cms.pop(id(pool)).__exit__(None, None, None)

    # ---------------- pools ----------------
    # LEFT stack: misc | x (stream rotation) | kpT | vp | qpT
    # RIGHT stack: stage | cst (bounce staging) | weights | eT
    misc = opn(name="misc", bufs=1, side="left")
    xp = opn(name="xp", bufs=1, side="left")
    stage = opn(name="stage", bufs=2, side="right")
    cstp = opn(name="cstp", bufs=1, side="right")
    wp = opn(name="wp", bufs=1, side="right")
    psmm = opn(name="psmm", bufs=6, space="PSUM")
    psacc = opn(name="psacc", bufs=1, space="PSUM")

    ones_f32 = misc.tile([P, P], f32, tag="ones_f32")
    nc.vector.memset(ones_f32[:], 1.0)
    ones = misc.tile([P, P], mm_dt, tag="ones")
    nc.vector.tensor_copy(ones[:], ones_f32[:])
    recip = misc.tile([P, HALF], f32, tag="recip")
    cs_ps = [psacc.tile([P, NF], f32, tag=f"cs{i}", name=f"cs{i}") for i in range(ISL)]

    # x-pool rotation: per-d stream chunks [P, NF], 2 slots.
    # Allocation order per d: k[0..1], vs[0..1], q[0..1]; each allocation
    # is emitted only after the slot's previous tenant has no more readers.
    def x_tile(kind, d, idx):
        return xp.tile([P, NF], mm_dt, tag=f"x{d}", bufs=2, name=f"{kind}{idx}_d{d}")

    def cst_tile(nm):
        return cstp.tile([P, NF], mm_dt, tag="cst", bufs=4, name=nm)

    # ---------------- PE warm-up while first DMAs land ----------------
    warm_ps = psmm.tile([P, P], f32, tag="mm", name="warm_ps")
    for _ in range(72):
        nc.tensor.matmul(warm_ps[:], ones[:], ones[:], start=True, stop=True)

    # weights: wk in two [P,NF] chunks (lower first-MM latency), wv/wq as
    # single [P,DK] rows. Static tags, no slot rotation.
    EPC = NF // P   # e-tiles per wk chunk
    WH = ET // EPC  # wk chunks per d
    wk_c = [[None] * WH for _ in range(DT)]
    wv_c = [None] * DT
    wq_c = [None] * DT

    k_c = [[None] * JSLH for _ in range(DT)]
    vs_c = [[None] * JGN for _ in range(DT)]
    q_c = [[None] * ISL for _ in range(DT)]

    # sync-ring descriptor order == consumption order:
    # wk.h0, k0, wk.h1, k1, wv, vs0 | vs1 | wq, q0 | q1 (bars = emit points
    # constrained by x-slot reuse; see loop bodies below)
    for d in range(DT):
        t = wp.tile([P, NF], mm_dt, tag=f"wk{d}h0", name=f"wk{d}_0")
        nc.sync.dma_start(t[:], wkT[ts(d, P), ts(0, NF)])
        wk_c[d][0] = t
    for d in range(DT):
        k_c[d][0] = x_tile("k", d, 0)
        nc.sync.dma_start(k_c[d][0][:], kTh[ts(d, P), ts(0, NF)])
    for d in range(DT):
        t = wp.tile([P, NF], mm_dt, tag=f"wk{d}h1", name=f"wk{d}_1")
        nc.sync.dma_start(t[:], wkT[ts(d, P), ts(1, NF)])
        wk_c[d][1] = t
    for d in range(DT):
        k_c[d][1] = x_tile("k", d, 1)
        nc.sync.dma_start(k_c[d][1][:], kTh[ts(d, P), ts(1, NF)])
    for d in range(DT):
        t = wp.tile([P, DK], mm_dt, tag=f"wv{d}", name=f"wv{d}")
        nc.sync.dma_start(t[:], wvT[ts(d, P), :])
        wv_c[d] = t

    def w_slice(c, d, e):
        return c[d][e // EPC][:, ts(e % EPC, P)]

    # ---------------- phase A: local kpT half -> bounce -> AllGather ----
    kp_pool = opn(name="kpp", bufs=1, side="left")
    kpT = [kp_pool.tile([P, S], mm_dt, tag=f"kp{e}", name=f"kp{e}") for e in range(ET)]

    for js in range(JSLH):
        for e in range(ET):
            ps = psmm.tile([P, NF], f32, tag="mm")
            for d in range(DT):
                nc.tensor.matmul(
                    ps[:],
                    w_slice(wk_c, d, e),
                    k_c[d][js][:],
                    start=(d == 0),
                    stop=(d == DT - 1),
                )
            st = cst_tile(f"kpb{js}_{e}")
            nc.vector.tensor_copy(st[:], ps[:])
            nc.scalar.dma_start(kp_in[js][ts(e, P), :], st[:])
        # vs chunk js reuses the k slot freed by this js slab
        for d in range(DT):
            vs_c[d][js] = x_tile("vs", d, js)
            nc.sync.dma_start(vs_c[d][js][:], vTh[ts(d, P), ts(js, NF)])
        nc.gpsimd.collective_compute(
            "AllGather",
            mybir.AluOpType.bypass,
            replica_groups=_PAIRS,
            ins=[kp_in[js][:, :]],
            outs=[kp_out[js][:, :]],
        )

    # wq loads ride the sync ring between the vs and q streams
    for d in range(DT):
        t = wp.tile([P, DK], mm_dt, tag=f"wq{d}", name=f"wq{d}")
        nc.sync.dma_start(t[:], wqT[ts(d, P), :])
        wq_c[d] = t

    # ---------------- phase E: local vp half -> bounce -> AllGather ----
    vp_pool = opn(name="vpp", bufs=1, side="left")
    vp = [vp_pool.tile([P, DK], mm_dt, tag=f"vp{j}", name=f"vp{j}") for j in range(JT)]

    for g in range(JGN):
        for jin in range(JPG):
            for es in range(ESL):
                ps = psmm.tile([P, NF], f32, tag="mm")
                for d in range(DT):
                    nc.tensor.matmul(
                        ps[:],
                        vs_c[d][g][:, ts(jin, P)],
                        wv_c[d][:, ts(es, NF)],
                        start=(d == 0),
                        stop=(d == DT - 1),
                    )
                st = cst_tile(f"vpb{g}_{jin}_{es}")
                nc.vector.tensor_copy(st[:], ps[:])
                nc.scalar.dma_start(vp_in[g][ts(jin, P), ts(es, NF)], st[:])
        # q slice g reuses the vs slot freed by this group
        for d in range(DT):
            q_c[d][g] = x_tile("q", d, g)
            nc.sync.dma_start(q_c[d][g][:], qT[ts(d, P), ts(g, NF)])
        nc.gpsimd.collective_compute(
            "AllGather",
            mybir.AluOpType.bypass,
            replica_groups=_PAIRS,
            ins=[vp_in[g][:, :]],
            outs=[vp_out[g][:, :]],
        )

    # kp readback on the sync ring after every input stream is queued
    # (these wait on the collectives; nothing consumption-critical queues
    # behind them). Both halves land in global j-order.
    for js in range(JSLH):
        for hr in range(2):
            for e in range(ET):
                nc.sync.dma_start(
                    kpT[e][:, ts(hr * JSLH + js, NF)],
                    kp_out[js][ts(hr * ET + e, P), :],
                )

    # ---------------- phase B: qpT = (q @ wq.T).T ----------------
    qp_pool = opn(name="qpp", bufs=1, side="left")
    qpT = [
        qp_pool.tile([P, HALF], mm_dt, tag=f"qp{e}", name=f"qp{e}") for e in range(ET)
    ]
    for isl in range(ISL):
        for e in range(ET):
            ps = psmm.tile([P, NF], f32, tag="mm")
            for d in range(DT):
                nc.tensor.matmul(
                    ps[:],
                    wq_c[d][:, ts(e, P)],
                    q_c[d][isl][:],
                    start=(d == 0),
                    stop=(d == DT - 1),
                )
            nc.vector.tensor_copy(qpT[e][:, ts(isl, NF)], ps[:])

    # vp readback (gated on the vp collectives, done well before F)
    for g in range(JGN):
        for hr in range(2):
            for jin in range(JPG):
                j = hr * (JT // 2) + g * JPG + jin
                nc.sync.dma_start(vp[j][:, :], vp_out[g][ts(hr * JPG + jin, P), :])

    # ---------------- phase C: sT -> exp -> eT (SBUF) + trailing colsum ----
    et_pool = opn(name="etp", bufs=1, side="right")
    eT = [et_pool.tile([P, HALF], mm_dt, tag=f"et{j}", name=f"et{j}") for j in range(JT)]
    pending_cs = []
    for j in range(JT):
        for isl in range(ISL):
            ps = psmm.tile([P, NF], f32, tag="mm")
            for e in range(ET):
                nc.tensor.matmul(
                    ps[:],
                    kpT[e][:, ts(j, P)],
                    qpT[e][:, ts(isl, NF)],
                    start=(e == 0),
                    stop=(e == ET - 1),
                )
            nc.scalar.activation(eT[j][:, ts(isl, NF)], ps[:], AF.Exp, scale=NORM)
            pending_cs.append((j, isl))
            if len(pending_cs) > 1:
                pj, pisl = pending_cs.pop(0)
                nc.tensor.matmul(
                    cs_ps[pisl][:],
                    ones[:],
                    eT[pj][:, ts(pisl, NF)],
                    start=(pj == 0),
                    stop=(pj == JT - 1),
                )
    for pj, pisl in pending_cs:
        nc.tensor.matmul(
            cs_ps[pisl][:],
            ones[:],
            eT[pj][:, ts(pisl, NF)],
            start=(pj == 0),
            stop=(pj == JT - 1),
        )
    for isl in range(ISL):
        nc.vector.reciprocal(recip[:, ts(isl, NF)], cs_ps[isl][:])

    # ---------------- phase F: outT = (eT.T @ vp).T * recip ----------------
    # Accumulators come from the same rotating psmm pool (a fresh PSUM pool
    # would wait on the colsum/reciprocal chain before its first bank frees).
    OH = NF // 2  # output DMA split: halves the tail after the last chain
    for isl in range(ISL):
        for e in range(ET):
            pft = psmm.tile([P, NF], f32, tag="mm", name=f"pf{e}_{isl}")
            for j in range(JT):
                nc.tensor.matmul(
                    pft[:],
                    vp[j][:, ts(e, P)],
                    eT[j][:, ts(isl, NF)],
                    start=(j == 0),
                    stop=(j == JT - 1),
                )
            ot = stage.tile([P, NF], f32, tag="ost")
            nc.vector.tensor_mul(ot[:], pft[:], recip[:, ts(isl, NF)])
            for u in range(2):
                nc.scalar.dma_start(
                    outT[ts(e, P), ts(isl * 2 + u, OH)],
                    ot[:, ts(u, OH)],
                )
    cls(qp_pool, vp_pool, kp_pool, xp, misc)
    cls(et_pool, wp, cstp, stage)
    cls(psacc)
    cls(psmm)


def build_program(DK=_DK, S=_S, HALF=_HALF, mm_dtype="bfloat16"):
    """Build + compile the per-core Bass program. Returns the Bacc object."""
    import concourse.tile as tile
    from concourse import bacc, mybir

    f32 = mybir.dt.float32
    mm_dt = getattr(mybir.dt, mm_dtype)
    KH = S // 2
    NF = 512

    nc = bacc.Bacc(
        "TRN2",
        target_bir_lowering=False,
        debug=False,
        enable_asserts=False,
        num_devices=_N_CORES,
    )
    qT = nc.dram_tensor("qt", (DK, HALF), mm_dt, kind="ExternalInput").ap()
    kTh = nc.dram_tensor("kt", (DK, KH), mm_dt, kind="ExternalInput").ap()
    vTh = nc.dram_tensor("vt", (DK, KH), mm_dt, kind="ExternalInput").ap()
    wqT = nc.dram_tensor("wqt", (DK, DK), mm_dt, kind="ExternalInput").ap()
    wkT = nc.dram_tensor("wkt", (DK, DK), mm_dt, kind="ExternalInput").ap()
    wvT = nc.dram_tensor("wvt", (DK, DK), mm_dt, kind="ExternalInput").ap()
    outT = nc.dram_tensor("outt", (DK, HALF), f32, kind="ExternalOutput").ap()

    # HBM bounce buffers for the pair AllGathers (2 chunks per tensor so
    # each collective kicks off as soon as its half-slab is produced)
    kp_in = [
        nc.dram_tensor(f"kp_in{c}", (DK, NF), mm_dt, kind="Internal").ap()
        for c in range(KH // NF)
    ]
    kp_out = [
        nc.dram_tensor(f"kp_out{c}", (2 * DK, NF), mm_dt, kind="Internal").ap()
        for c in range(KH // NF)
    ]
    vp_in = [
        nc.dram_tensor(f"vp_in{g}", (KH // 2, DK), mm_dt, kind="Internal").ap()
        for g in range(KH // NF)
    ]
    vp_out = [
        nc.dram_tensor(f"vp_out{g}", (KH, DK), mm_dt, kind="Internal").ap()
        for g in range(KH // NF)
    ]

    with tile.TileContext(nc) as tc:
        _emit(
            tc,
            qT,
            kTh,
            vTh,
            wqT,
            wkT,
            wvT,
            outT,
            (kp_in, kp_out, vp_in, vp_out),
            DK,
            S,
            HALF,
            mm_dt,
        )
    nc.compile()
    return nc


def _in_maps(q, k, v, wq, wk, wv):
    """Shard full inputs into 8 per-core input maps (host-side transposes)."""
    import ml_dtypes

    bf16 = ml_dtypes.bfloat16
    wqT = np.ascontiguousarray(wq.T).astype(bf16)
    wkT = np.ascontiguousarray(wk.T).astype(bf16)
    wvT = np.ascontiguousarray(wv.T).astype(bf16)
    maps = []
    for c in range(_N_CORES):
        b, h = divmod(c, 2)
        sl = slice(h * _HALF, (h + 1) * _HALF)
        maps.append(
            {
                "qt": np.ascontiguousarray(q[b, sl, :].T).astype(bf16),
                "kt": np.ascontiguousarray(k[b, sl, :].T).astype(bf16),
                "vt": np.ascontiguousarray(v[b, sl, :].T).astype(bf16),
                "wqt": wqT,
                "wkt": wkT,
                "wvt": wvT,
            }
        )
    return maps


def kernel(q, k, v, wq, wk, wv):
    from concourse.bass_utils import run_bass_kernel_spmd

    q = np.asarray(q, np.float32)
    k = np.asarray(k, np.float32)
    v = np.asarray(v, np.float32)
    wq = np.asarray(wq, np.float32)
    wk = np.asarray(wk, np.float32)
    wv = np.asarray(wv, np.float32)

    if "nc" not in _CACHE:
        _CACHE["nc"] = build_program()
    nc = _CACHE["nc"]

    res = run_bass_kernel_spmd(
        nc, _in_maps(q, k, v, wq, wk, wv), core_ids=list(range(_N_CORES))
    )

    out = np.empty((_B, _S, _DK), np.float32)
    for c in range(_N_CORES):
        b, h = divmod(c, 2)
        out[b, h * _HALF : (h + 1) * _HALF, :] = res.results[c]["outt"].T
    return out


# revision 14
# speedup vs baseline: 1.3057x; 1.1688x over previous
"""Trainium2 Bass kernel for nn_AttentionHead (B=4, S=2048, DK=1024).

Single-head attention with input projections:
    qp = q @ wq.T; kp = k @ wk.T; vp = v @ wv.T
    s  = qp @ kp.T / sqrt(dk); attn = softmax(s); out = attn @ vp

Sharding: 8 cores = (batch b in 0..3) x (sequence half h in 0..1).
Each core owns 1024 query rows AND 1024 key/value rows of its batch.
K/V projections are computed once per row (no duplication across the
pair): each core projects only its own 1024 k/v rows, then the pair
exchanges halves with 2-rank AllGathers through HBM bounce buffers
(SPMD-uniform layout: both halves are read back from the AllGather
output in global j-order, so the program is identical on all cores).

Per core (all matmul operands bf16, fp32 PSUM accumulation):
    A: kpT_loc[e,j'] = sum_d wkT[d,e] kT_loc[d,j']   (128 MMs)
       -> bounce -> AllGather(pair) -> kpT[e, 0:2048]
    E: vp_loc[j',e]  = sum_d vT_loc[d,j'] wvT[d,e]   (128 MMs)
       -> bounce -> AllGather(pair) -> vp[j] for all 16 j-tiles
    B: qpT[e,i]  = sum_d wqT[d,e] qT[d,i]            (128 MMs)
    C: sT[j,i]   = sum_e kpT[e,j] qpT[e,i]           (256 MMs)
       eT[j,i]   = exp(sT/32)  (ACT, fused scale, stays in SBUF)
       cs[i]     = sum_j eT[j,i] via ones-matmul      (32 MMs, trailing)
    F: outT[e,i] = (sum_j vp[j,e] eT[j,i]) * (1/cs[i])  (256 MMs)

DMA-descriptor generation is the hidden serial resource: each
dma_start costs ~0.6us of descriptor generation on its issuing
engine's sequencer, strictly in program order. The ~200 DMAs are
split across BOTH hardware DGE rings so neither backs up: input
streams + AllGather readbacks on the sync ring (consumption order),
bounce writes + output stores on the scalar ring. Collectives
trigger from gpsimd. Each ring's program order equals it
consumption order, so no head-of-line inversion is possible.

Phase-F accumulators come from the same rotating 6-bank PSUM pool
as the earlier phases (no pool close/open at the C->F boundary --
a fresh pool would wait on the colsum-reciprocal chain and stall
the PE ~6us). The reciprocal runs on the scalar engine.

928 matmuls/core at ~216 ns each (bf16; FWL + LDWEIGHTS prefetch
hide the weight-load bubble). Measured end-to-end relative error vs
the fp32 reference: ~6e-3 (bf16 quantization).
"""

import numpy as np

_B, _S, _DK = 4, 2048, 1024
_HALF = _S // 2
_N_CORES = 8
_P = 128
_PAIRS = [[0, 1], [2, 3], [4, 5], [6, 7]]

_CACHE = {}


def _emit(tc, qT, kTh, vTh, wqT, wkT, wvT, outT, cc, DK, S, HALF, mm_dt):
    import concourse.bass as bass
    from concourse import mybir

    nc = tc.nc
    ts = bass.ts
    P = _P
    NF = 512
    KH = S // 2            # local key/value rows
    DT = DK // P           # contraction tiles (d)
    ET = DK // P           # output-feature tiles (e)
    JT = S // P            # global key tiles (j)
    ISL = HALF // NF       # query slices (i)
    JSLH = KH // NF        # local key slices
    ESL = DK // NF         # feature slices
    JGN = KH // NF         # local vT chunk groups
    JPG = NF // P          # j-tiles per vT chunk
    NORM = 1.0 / float(np.sqrt(DK))
    f32 = mybir.dt.float32
    AF = mybir.ActivationFunctionType
    kp_in, kp_out, vp_in, vp_out = cc

    _cms = {}

    def opn(**kw):
        cm = tc.tile_pool(**kw)
        pool = cm.__enter__()
        _cms[id(pool)] = cm
        return pool

    def cls(*pools):
        for pool in pools:
            _cms.pop(id(pool)).__exit__(None, None, None)

    # ---------------- pools ----------------
    # LEFT stack: misc | x (stream rotation) | kpT | vp | qpT
    # RIGHT stack: stage | cst (bounce staging) | weights | eT
    misc = opn(name="misc", bufs=1, side="left")
    xp = opn(name="xp", bufs=1, side="left")
    stage = opn(name="stage", bufs=2, side="right")
    cstp = opn(name="cstp", bufs=1, side="right")
    wp = opn(name="wp", bufs=1, side="right")
    psmm = opn(name="psmm", bufs=6, space="PSUM")
    psacc = opn(name="psacc", bufs=1, space="PSUM")

    ones_f32 = misc.tile([P, P], f32, tag="ones_f32")
    nc.vector.memset(ones_f32[:], 1.0)
    ones = misc.tile([P, P], mm_dt, tag="ones")
    nc.vector.tensor_copy(ones[:], ones_f32[:])
    recip = misc.tile([P, HALF], f32, tag="recip")
    cs_ps = [psacc.tile([P, NF], f32, tag=f"cs{i}", name=f"cs{i}") for i in range(ISL)]

    # x-pool rotation: per-d stream chunks [P, NF], 2 slots.
    # Allocation order per d: k[0..1], vs[0..1]; each allocation is
    # emitted only after the slot's previous tenant has no more readers.
    # q rides the second buffer of the wv tag (no slot-wait: its transfers
    # finish before the AllGather readback floods hit the DMA engines).
    def x_tile(kind, d, idx):
        return xp.tile([P, NF], mm_dt, tag=f"x{d}", bufs=2, name=f"{kind}{idx}_d{d}")

    def cst_tile(nm):
        return cstp.tile([P, NF], mm_dt, tag="cst", bufs=8, name=nm)

    # ---------------- PE warm-up while first DMAs land ----------------
    warm_ps = psmm.tile([P, P], f32, tag="mm", name="warm_ps")
    for _ in range(100):
        nc.tensor.matmul(warm_ps[:], ones[:], ones[:], start=True, stop=True)

    # weights: wk in two [P,NF] chunks (lower first-MM latency), wv/wq as
    # single [P,DK] rows. Static tags, no slot rotation.
    EPC = NF // P   # e-tiles per wk chunk
    WH = ET // EPC  # wk chunks per d
    wk_c = [[None] * WH for _ in range(DT)]
    wv_c = [None] * DT
    wq_c = [None] * DT

    k_c = [[None] * JSLH for _ in range(DT)]
    vs_c = [[None] * JGN for _ in range(DT)]
    q_c = [None] * DT

    # sync-ring descriptor order == consumption order:
    # wk.h0, k0, wk.h1, k1, wv, vs0 | vs1 | wq, q0 | q1 (bars = emit points
    # constrained by x-slot reuse; see loop bodies below)
    for d in range(DT):
        t = wp.tile([P, NF], mm_dt, tag=f"wk{d}h0", name=f"wk{d}_0")
        nc.sync.dma_start(t[:], wkT[ts(d, P), ts(0, NF)])
        wk_c[d][0] = t
    for d in range(DT):
        k_c[d][0] = x_tile("k", d, 0)
        nc.sync.dma_start(k_c[d][0][:], kTh[ts(d, P), ts(0, NF)])
    for d in range(DT):
        t = wp.tile([P, NF], mm_dt, tag=f"wk{d}h1", name=f"wk{d}_1")
        nc.sync.dma_start(t[:], wkT[ts(d, P), ts(1, NF)])
        wk_c[d][1] = t
    for d in range(DT):
        k_c[d][1] = x_tile("k", d, 1)
        nc.sync.dma_start(k_c[d][1][:], kTh[ts(d, P), ts(1, NF)])
    for d in range(DT):
        t = wp.tile([P, DK], mm_dt, tag=f"wv{d}", bufs=2, name=f"wv{d}")
        nc.sync.dma_start(t[:], wvT[ts(d, P), :])
        wv_c[d] = t

    def w_slice(c, d, e):
        return c[d][e // EPC][:, ts(e % EPC, P)]

    # ---------------- phase A: local kpT half -> bounce -> AllGather ----
    kp_pool = opn(name="kpp", bufs=1, side="left")
    kpT = [kp_pool.tile([P, S], mm_dt, tag=f"kp{e}", name=f"kp{e}") for e in range(ET)]

    for js in range(JSLH):
        for e in range(ET):
            ps = psmm.tile([P, NF], f32, tag="mm")
            for d in range(DT):
                nc.tensor.matmul(
                    ps[:],
                    w_slice(wk_c, d, e),
                    k_c[d][js][:],
                    start=(d == 0),
                    stop=(d == DT - 1),
                )
            st = cst_tile(f"kpb{js}_{e}")
            nc.vector.tensor_copy(st[:], ps[:])
            nc.scalar.dma_start(kp_in[ts(e, P), ts(js, NF)], st[:])
        # vs chunk js reuses the k slot freed by this js slab
        for d in range(DT):
            vs_c[d][js] = x_tile("vs", d, js)
            nc.sync.dma_start(vs_c[d][js][:], vTh[ts(d, P), ts(js, NF)])
    nc.gpsimd.collective_compute(
        "AllGather",
        mybir.AluOpType.bypass,
        replica_groups=_PAIRS,
        ins=[kp_in[:, :]],
        outs=[kp_out[:, :]],
    )

    # wq + q loads ride the sync ring behind the vs streams; q uses the
    # spare wv buffer so its transfers have no slot-wait and land before
    # the readback floods
    for d in range(DT):
        t = wp.tile([P, DK], mm_dt, tag=f"wq{d}", name=f"wq{d}")
        nc.sync.dma_start(t[:], wqT[ts(d, P), :])
        wq_c[d] = t
    for d in range(DT):
        t = wp.tile([P, DK], mm_dt, tag=f"wv{d}", bufs=2, name=f"q{d}")
        nc.sync.dma_start(t[:], qT[ts(d, P), :])
        q_c[d] = t

    # ---------------- phase E: local vp half -> bounce -> AllGather ----
    vp_pool = opn(name="vpp", bufs=1, side="left")
    vp = [vp_pool.tile([P, DK], mm_dt, tag=f"vp{j}", name=f"vp{j}") for j in range(JT)]

    for g in range(JGN):
        for jin in range(JPG):
            for es in range(ESL):
                ps = psmm.tile([P, NF], f32, tag="mm")
                for d in range(DT):
                    nc.tensor.matmul(
                        ps[:],
                        vs_c[d][g][:, ts(jin, P)],
                        wv_c[d][:, ts(es, NF)],
                        start=(d == 0),
                        stop=(d == DT - 1),
                    )
                st = cst_tile(f"vpb{g}_{jin}_{es}")
                nc.vector.tensor_copy(st[:], ps[:])
                nc.scalar.dma_start(
                    vp_in[ts(g * JPG + jin, P), ts(es, NF)], st[:]
                )
    nc.gpsimd.collective_compute(
        "AllGather",
        mybir.AluOpType.bypass,
        replica_groups=_PAIRS,
        ins=[vp_in[:, :]],
        outs=[vp_out[:, :]],
    )

    # kp readback on the sync ring after every input stream is queued
    # (these wait on the collective; nothing consumption-critical queues
    # behind them). Both halves land in global j-order.
    for hr in range(2):
        for e in range(ET):
            nc.sync.dma_start(
                kpT[e][:, ts(hr, KH)],
                kp_out[ts(hr * ET + e, P), :],
            )

    # ---------------- phase B: qpT = (q @ wq.T).T ----------------
    qp_pool = opn(name="qpp", bufs=1, side="left")
    qpT = [
        qp_pool.tile([P, HALF], mm_dt, tag=f"qp{e}", name=f"qp{e}") for e in range(ET)
    ]
    for isl in range(ISL):
        for e in range(ET):
            ps = psmm.tile([P, NF], f32, tag="mm")
            for d in range(DT):
                nc.tensor.matmul(
                    ps[:],
                    wq_c[d][:, ts(e, P)],
                    q_c[d][:, ts(isl, NF)],
                    start=(d == 0),
                    stop=(d == DT - 1),
                )
            nc.vector.tensor_copy(qpT[e][:, ts(isl, NF)], ps[:])

    # vp readback (gated on the vp collective, done well before F)
    for hr in range(2):
        for jl in range(JT // 2):
            nc.sync.dma_start(
                vp[hr * (JT // 2) + jl][:, :], vp_out[ts(hr * (JT // 2) + jl, P), :]
            )

    # weights are dead after B; free their SBUF so eT can live there
    cls(wp)

    # ---------------- phase C: sT -> exp -> eT (SBUF) + trailing colsum ----
    et_pool = opn(name="etp", bufs=1, side="right")
    eT = [et_pool.tile([P, HALF], mm_dt, tag=f"et{j}", name=f"et{j}") for j in range(JT)]
    pending_cs = []
    for j in range(JT):
        for isl in range(ISL):
            ps = psmm.tile([P, NF], f32, tag="mm")
            for e in range(ET):
                nc.tensor.matmul(
                    ps[:],
                    kpT[e][:, ts(j, P)],
                    qpT[e][:, ts(isl, NF)],
                    start=(e == 0),
                    stop=(e == ET - 1),
                )
            nc.scalar.activation(eT[j][:, ts(isl, NF)], ps[:], AF.Exp, scale=NORM)
            pending_cs.append((j, isl))
            if len(pending_cs) > 1:
                pj, pisl = pending_cs.pop(0)
                nc.tensor.matmul(
                    cs_ps[pisl][:],
                    ones[:],
                    eT[pj][:, ts(pisl, NF)],
                    start=(pj == 0),
                    stop=(pj == JT - 1),
                )
    for pj, pisl in pending_cs:
        nc.tensor.matmul(
            cs_ps[pisl][:],
            ones[:],
            eT[pj][:, ts(pisl, NF)],
            start=(pj == 0),
            stop=(pj == JT - 1),
        )
    for isl in range(ISL):
        nc.vector.reciprocal(recip[:, ts(isl, NF)], cs_ps[isl][:])

    # ---------------- phase F: outT = (eT.T @ vp).T * recip ----------------
    # Accumulators come from the same rotating psmm pool (a fresh PSUM pool
    # would wait on the colsum/reciprocal chain before its first bank frees).
    OH = NF // 2  # output DMA split: halves the tail after the last chain
    for isl in range(ISL):
        for e in range(ET):
            pft = psmm.tile([P, NF], f32, tag="mm", name=f"pf{e}_{isl}")
            for j in range(JT):
                nc.tensor.matmul(
                    pft[:],
                    vp[j][:, ts(e, P)],
                    eT[j][:, ts(isl, NF)],
                    start=(j == 0),
                    stop=(j == JT - 1),
                )
            ot = stage.tile([P, NF], f32, tag="ost")
            nc.vector.tensor_mul(ot[:], pft[:], recip[:, ts(isl, NF)])
            for u in range(2):
                nc.scalar.dma_start(
                    outT[ts(e, P), ts(isl * 2 + u, OH)],
                    ot[:, ts(u, OH)],
                )
    cls(qp_pool, vp_pool, kp_pool, xp, misc)
    cls(et_pool, cstp, stage)
    cls(psacc)
    cls(psmm)


def build_program(DK=_DK, S=_S, HALF=_HALF, mm_dtype="bfloat16"):
    """Build + compile the per-core Bass program. Returns the Bacc object."""
    import concourse.tile as tile
    from concourse import bacc, mybir

    f32 = mybir.dt.float32
    mm_dt = getattr(mybir.dt, mm_dtype)
    KH = S // 2
    NF = 512

    nc = bacc.Bacc(
        "TRN2",
        target_bir_lowering=False,
        debug=False,
        enable_asserts=False,
        num_devices=_N_CORES,
    )
    qT = nc.dram_tensor("qt", (DK, HALF), mm_dt, kind="ExternalInput").ap()
    kTh = nc.dram_tensor("kt", (DK, KH), mm_dt, kind="ExternalInput").ap()
    vTh = nc.dram_tensor("vt", (DK, KH), mm_dt, kind="ExternalInput").ap()
    wqT = nc.dram_tensor("wqt", (DK, DK), mm_dt, kind="ExternalInput").ap()
    wkT = nc.dram_tensor("wkt", (DK, DK), mm_dt, kind="ExternalInput").ap()
    wvT = nc.dram_tensor("wvt", (DK, DK), mm_dt, kind="ExternalInput").ap()
    outT = nc.dram_tensor("outt", (DK, HALF), f32, kind="ExternalOutput").ap()

    # HBM bounce buffers for the pair AllGathers (one per tensor: each
    # collective carries a ~25us firmware latency floor, so fewer is faster)
    kp_in = nc.dram_tensor("kp_in", (DK, KH), mm_dt, kind="Internal").ap()
    kp_out = nc.dram_tensor("kp_out", (2 * DK, KH), mm_dt, kind="Internal").ap()
    vp_in = nc.dram_tensor("vp_in", (KH, DK), mm_dt, kind="Internal").ap()
    vp_out = nc.dram_tensor("vp_out", (2 * KH, DK), mm_dt, kind="Internal").ap()

    with tile.TileContext(nc) as tc:
        _emit(
            tc,
            qT,
            kTh,
            vTh,
            wqT,
            wkT,
            wvT,
            outT,
            (kp_in, kp_out, vp_in, vp_out),
            DK,
            S,
            HALF,
            mm_dt,
        )
    nc.compile()
    return nc


def _in_maps(q, k, v, wq, wk, wv):
    """Shard full inputs into 8 per-core input maps (host-side transposes)."""
    import ml_dtypes

    bf16 = ml_dtypes.bfloat16
    wqT = np.ascontiguousarray(wq.T).astype(bf16)
    wkT = np.ascontiguousarray(wk.T).astype(bf16)
    wvT = np.ascontiguousarray(wv.T).astype(bf16)
    maps = []
    for c in range(_N_CORES):
        b, h = divmod(c, 2)
        sl = slice(h * _HALF, (h + 1) * _HALF)
        maps.append(
            {
                "qt": np.ascontiguousarray(q[b, sl, :].T).astype(bf16),
                "kt": np.ascontiguousarray(k[b, sl, :].T).astype(bf16),
                "vt": np.ascontiguousarray(v[b, sl, :].T).astype(bf16),
                "wqt": wqT,
                "wkt": wkT,
                "wvt": wvT,
            }
        )
    return maps


def kernel(q, k, v, wq, wk, wv):
    from concourse.bass_utils import run_bass_kernel_spmd

    q = np.asarray(q, np.float32)
    k = np.asarray(k, np.float32)
    v = np.asarray(v, np.float32)
    wq = np.asarray(wq, np.float32)
    wk = np.asarray(wk, np.float32)
    wv = np.asarray(wv, np.float32)

    if "nc" not in _CACHE:
        _CACHE["nc"] = build_program()
    nc = _CACHE["nc"]

    res = run_bass_kernel_spmd(
        nc, _in_maps(q, k, v, wq, wk, wv), core_ids=list(range(_N_CORES))
    )

    out = np.empty((_B, _S, _DK), np.float32)
    for c in range(_N_CORES):
        b, h = divmod(c, 2)
        out[b, h * _HALF : (h + 1) * _HALF, :] = res.results[c]["outt"].T
    return out
